# revision 1
# baseline (speedup 1.0000x reference)
"""Distributed Trainium2 kernel for nn_Attention_61332132987140.

Gated multi-head attention block: RMSNorm -> QKV proj -> RoPE -> softmax
attention -> sigmoid head gating -> output projection.

Sharding: 8 cores = 2 batch groups x 4-head groups (tensor parallel on
heads within a batch). Each core computes attention for its batch's full
sequence over its 4 heads, the partial output projection over its 256
columns of w_out, then a ReduceScatter over its 4-core batch group sums
the partials and leaves each core with a disjoint 128-token slice per
512-token quarter. The host reassembles the full (2, 2048, 1024) output.

Device compute dtype: bf16 operands into the PE array with fp32 PSUM
accumulation; softmax/normalization math in fp32 on ACT/DVE.
"""
import os
import sys

sys.path.insert(0, "/opt/trn_rl_repo")

import numpy as np
import ml_dtypes

import concourse.bass as bass
import concourse.mybir as mybir
import concourse.tile as tile
from concourse import bacc
from concourse.bass_utils import run_bass_kernel_spmd

F32 = mybir.dt.float32
BF16 = mybir.dt.bfloat16
AF = mybir.ActivationFunctionType
ALU = mybir.AluOpType

B, N, DIM = 2, 2048, 1024
HEADS, DH = 16, 64
HL = 4  # local heads per core
P = 128
TT = N // P  # 16 token tiles
KD = DIM // P  # 8 contraction tiles
NQ = 4  # quarters (512-token i-chunks)
QT = N // NQ
CORES = 8
REPLICA_GROUPS = [[0, 1, 2, 3], [4, 5, 6, 7]]

_nc_cache = None
_last_result = None


def _build():
    nc = bacc.Bacc("TRN2", target_bir_lowering=False, debug=False, num_devices=CORES)

    x_ext = nc.declare_dram_parameter("x", [N, DIM], F32, isOutput=False)
    wqkv_ext = nc.declare_dram_parameter("wqkv", [DIM, 3 * HL * DH], BF16, isOutput=False)
    wg_ext = nc.declare_dram_parameter("wg", [DIM, HL], BF16, isOutput=False)
    bgn_ext = nc.declare_dram_parameter("bgn", [HL, 1], F32, isOutput=False)
    wout_ext = nc.declare_dram_parameter("wout", [HL * DH, DIM], BF16, isOutput=False)
    cosr_ext = nc.declare_dram_parameter("cosr", [N, 512], BF16, isOutput=False)
    sinr_ext = nc.declare_dram_parameter("sinr", [N, 512], BF16, isOutput=False)
    out_ext = nc.declare_dram_parameter("out", [NQ, P, DIM], F32, isOutput=True)
    dbg = {}
    if os.environ.get("KDEBUG"):
        dbg["xnT"] = nc.declare_dram_parameter("dbg_xnT", [P, 4, KD, P], BF16,
                                               isOutput=True)
        dbg["qkt"] = nc.declare_dram_parameter("dbg_qkt", [P, TT, 4, P], BF16,
                                               isOutput=True)
        dbg["v"] = nc.declare_dram_parameter("dbg_v", [P, TT, HL * DH], BF16,
                                             isOutput=True)
        dbg["gates"] = nc.declare_dram_parameter("dbg_gates", [P, 512], F32,
                                                 isOutput=True)
        dbg["pt"] = nc.declare_dram_parameter("dbg_pt", [P, 2, 512], BF16,
                                              isOutput=True)
        dbg["sums"] = nc.declare_dram_parameter("dbg_sums", [P, 512], F32,
                                                isOutput=True)
        dbg["av"] = nc.declare_dram_parameter("dbg_av", [P, 512], F32,
                                              isOutput=True)
        dbg["oT"] = nc.declare_dram_parameter("dbg_oT", [P, 2, 512], BF16,
                                              isOutput=True)
        dbg["y"] = nc.declare_dram_parameter("dbg_y", [QT, DIM], F32,
                                             isOutput=True)

    with tile.TileContext(nc) as tc:
        with (
            tc.tile_pool(name="wpool", bufs=1) as wpool,
            tc.tile_pool(name="persist", bufs=1) as persist,
            tc.tile_pool(name="xstream", bufs=4) as xstream,
            tc.tile_pool(name="stream", bufs=3) as stream,
            tc.tile_pool(name="xntp", bufs=2) as xntp,
            tc.tile_pool(name="ptp", bufs=28) as ptp,
            tc.tile_pool(name="tail", bufs=2) as tailp,
            tc.tile_pool(name="ps_qa1", bufs=1, space="PSUM") as ps_qa1,
            tc.tile_pool(name="ps_qa2", bufs=1, space="PSUM") as ps_qa2,
            tc.tile_pool(name="ps_s", bufs=2, space="PSUM") as ps_s,
                        tc.tile_pool(name="ps_attn", bufs=1, space="PSUM") as ps_attn,
            tc.tile_pool(name="ps_ygs", bufs=2, space="PSUM") as ps_ygs,
            tc.tile_pool(name="dram", bufs=1, space="DRAM") as dramp,
        ):
            # ---- constants / weights ----
            wqkv_sb = wpool.tile([P, KD, 768], BF16)
            nc.scalar.dma_start(
                wqkv_sb[:], wqkv_ext.rearrange("(k p) f -> p k f", p=P)
            )
            wg_sb = wpool.tile([P, KD, HL], BF16)
            nc.scalar.dma_start(wg_sb[:], wg_ext.rearrange("(k p) f -> p k f", p=P))
            wout_sb = wpool.tile([P, 2, DIM], BF16)
            nc.scalar.dma_start(
                wout_sb[:], wout_ext.rearrange("(k p) f -> p k f", p=P)
            )
            bgn_sb = wpool.tile([HL, 1], F32)
            nc.scalar.dma_start(bgn_sb[:], bgn_ext[:])
            zb = wpool.tile([P, 1], F32)
            nc.gpsimd.memset(zb[:], 0.0)
            lb32 = wpool.tile([P, 1], F32)
            nc.gpsimd.memset(lb32[:], float(np.log(32.0)))
            ones_sb = wpool.tile([P, 1], BF16)
            nc.gpsimd.memset(ones_sb[:], 1.0)
            ones_row = wpool.tile([1, 64], BF16)
            nc.gpsimd.memset(ones_row[:], 1.0)

            # ---- persistent activations ----
            # QKT_sb[p, tok_tile, blk, t]: blk 0/1 = q head-pairs; 2/3 = k.
            # blk-last layout keeps each xbar-transpose destination dense
            QKT_sb = persist.tile([P, TT, 4, P], BF16)
            # v_sb[j_in_tile, jt, h*64+d]
            v_sb = persist.tile([P, TT, HL * DH], BF16)
            # gates for head h live at partition 32*h; other rows are garbage
            gates_sb = persist.tile([P, N], F32)

            def emit_se(ci, jt, dbg_tap=False):
                """scores + exp for one (quarter, j-tile); returns pt tiles"""
                pts = []
                for hp in range(2):
                    s_ps = ps_s.tile([P, 2, 512], F32, name="s_ps", tag="s")
                    for e in range(2):
                        nc.tensor.matmul(
                            s_ps[:, e, :],
                            QKT_sb[e * 64:(e + 1) * 64, jt, 2 + hp, :],
                            QKT_sb[e * 64:(e + 1) * 64, 4 * ci:4 * ci + 4,
                                   hp, :],
                            start=True, stop=True,
                        )
                    pt = ptp.tile([P, 2, 512], BF16, name="pt")
                    nc.scalar.activation(pt[:], s_ps[:], AF.Exp, scale=0.125,
                                         bias=zb[:])
                    pts.append(pt)
                    if dbg_tap and hp == 0:
                        nc.gpsimd.dma_start(dbg["pt"][:], pt[:])
                return pts

            def emit_avs(jt, pts, av01, av23, sums):
                """AV accumulation + softmax-sum matmuls for one j-tile"""
                for hp in range(2):
                    avt = av01 if hp == 0 else av23
                    # adjacent issue of col-disjoint AV matmuls -> concurrent.
                    # Concurrent accumulation groups in one bank are fine:
                    # partition ranges disjoint; has_written is per row
                    for e in range(2):
                        h = 2 * hp + e
                        nc.tensor.matmul(
                            avt[e * 64:(e + 1) * 64, :],
                            v_sb[:, jt, h * DH:(h + 1) * DH],
                            pts[hp][:, e, :],
                            start=(jt == 0), stop=(jt == TT - 1),
                            skip_group_check=True,
                        )
                # four sum matmuls back-to-back: distinct 32-col groups ->
                # one concurrent 512-cycle slot
                for hp in range(2):
                    for e in range(2):
                        h = 2 * hp + e
                        nc.tensor.matmul(
                            sums[h * 32:h * 32 + 1, :],
                            ones_sb[:, 0:1],
                            pts[hp][:, e, :],
                            start=(jt == 0), stop=(jt == TT - 1),
                            tile_position=(0, h * 32),
                            skip_group_check=True,
                        )

            # =========== Phase A: norm, QKV, RoPE, transposes ===========
            pre_pts = {}
            for ci in range(NQ):
                # xnT[p, tt, kd, t]: kd-last so each transpose dest is dense
                xnT = xntp.tile([P, 4, KD, P], BF16, name="xnT")
                # batch the 4 tiles' Ln/Exp into single ops: the ACT table
                # pass places a LoadActFuncSet at every Ln<->Exp alternation,
                # so grouping keeps it to 2 loads per chunk
                xts = []
                ss4 = stream.tile([P, 4], F32, name="ss4")
                for tt in range(4):
                    tok = ci * 4 + tt
                    x_t = xstream.tile([P, DIM], F32, name="x_t")
                    nc.gpsimd.dma_start(x_t[:], x_ext[tok * P:(tok + 1) * P, :])
                    xts.append(x_t)
                    scr = stream.tile([P, DIM], BF16, name="scr")
                    nc.vector.tensor_tensor(out=scr[:], in0=x_t[:], in1=x_t[:],
                                            op=ALU.mult)
                    nc.vector.reduce_sum(ss4[:, tt:tt + 1], scr[:],
                                         axis=mybir.AxisListType.X)
                ln4 = stream.tile([P, 4], F32, name="ln4")
                nc.scalar.activation(ln4[:], ss4[:], AF.Ln, bias=zb[:])
                sc4 = stream.tile([P, 4], F32, name="sc4")
                nc.scalar.activation(sc4[:], ln4[:], AF.Exp, scale=-0.5,
                                     bias=lb32[:])
                for tt in range(4):
                    xn_t = stream.tile([P, DIM], BF16, name="xn_t")
                    nc.scalar.activation(xn_t[:], xts[tt][:], AF.Copy,
                                         scale=sc4[:, tt:tt + 1])
                    nc.sync.dma_start_transpose(xnT[:, tt, :, :], xn_t[:])

                for tt in range(4):
                    tok = ci * 4 + tt
                    qk_ps = ps_qa1.tile([P, 512], F32, name="qk_ps", tag="qa1")
                    v_ps = ps_qa2.tile([P, 256], F32, name="v_ps", tag="qa2")
                    for kd in range(KD):
                        lhsT = xnT[:, tt, kd, :]
                        nc.tensor.matmul(qk_ps[:], lhsT,
                                         wqkv_sb[:, kd, 0:512],
                                         start=(kd == 0), stop=(kd == KD - 1))
                        nc.tensor.matmul(v_ps[:], lhsT,
                                         wqkv_sb[:, kd, 512:768],
                                         start=(kd == 0), stop=(kd == KD - 1))
                    # RoPE on q|k (psum cols 0:512), even/odd feature halves
                    cos_t = stream.tile([P, 512], BF16, name="cos_t")
                    nc.gpsimd.dma_start(cos_t[:], cosr_ext[tok * P:(tok + 1) * P, :])
                    sin_t = stream.tile([P, 512], BF16, name="sin_t")
                    nc.gpsimd.dma_start(sin_t[:], sinr_ext[tok * P:(tok + 1) * P, :])
                    qkv = qk_ps[:].rearrange("p (b c) -> p b c", b=8)
                    qE, qO = qkv[:, :, 0:32], qkv[:, :, 32:64]
                    cE = cos_t[:, 0:256].rearrange("p (b c) -> p b c", b=8)
                    cO = cos_t[:, 256:512].rearrange("p (b c) -> p b c", b=8)
                    sE = sin_t[:, 0:256].rearrange("p (b c) -> p b c", b=8)
                    sO = sin_t[:, 256:512].rearrange("p (b c) -> p b c", b=8)
                    t1 = stream.tile([P, 256], F32, name="t1")
                    t2 = stream.tile([P, 256], F32, name="t2")
                    t1v = t1[:].rearrange("p (b c) -> p b c", b=8)
                    t2v = t2[:].rearrange("p (b c) -> p b c", b=8)
                    qk_sb = stream.tile([P, 512], BF16, name="qk_sb")
                    qkv_out = qk_sb[:].rearrange("p (b c) -> p b c", b=8)
                    outE, outO = qkv_out[:, :, 0:32], qkv_out[:, :, 32:64]
                    nc.vector.tensor_tensor(out=t1v, in0=qE, in1=cE, op=ALU.mult)
                    nc.vector.tensor_tensor(out=t2v, in0=qO, in1=sE, op=ALU.mult)
                    nc.vector.tensor_tensor(out=outE, in0=t1v, in1=t2v,
                                            op=ALU.subtract)
                    t3 = stream.tile([P, 256], F32, name="t1")
                    t4 = stream.tile([P, 256], F32, name="t2")
                    t3v = t3[:].rearrange("p (b c) -> p b c", b=8)
                    t4v = t4[:].rearrange("p (b c) -> p b c", b=8)
                    nc.vector.tensor_tensor(out=t3v, in0=qO, in1=cO, op=ALU.mult)
                    nc.vector.tensor_tensor(out=t4v, in0=qE, in1=sO, op=ALU.mult)
                    nc.vector.tensor_tensor(out=outO, in0=t3v, in1=t4v, op=ALU.add)
                    # v: psum cols 512:768 -> v_sb
                    nc.vector.tensor_copy(v_sb[:, tok, :], v_ps[:])
                    # transpose rotated q|k into QKT
                    nc.sync.dma_start_transpose(QKT_sb[:, tok, :, :], qk_sb[:])

                # gates for this chunk: sigmoid(xn @ wg.T + b) via exp, then
                # scatter head h to partition 32*h of gates_sb (DVE operand
                # bases must be 32-aligned, DMA moves partitions freely)
                gates_ps = ps_ygs.tile([HL, 512], F32, name="gates_ps", tag="ygs")
                for kd in range(KD):
                    nc.tensor.matmul(gates_ps[:], wg_sb[:, kd, :],
                                     xnT[:, :, kd, :],
                                     start=(kd == 0), stop=(kd == KD - 1))
                ge = stream.tile([HL, 512], F32, name="ge")
                nc.scalar.activation(ge[:], gates_ps[:], AF.Exp, scale=-1.0,
                                     bias=bgn_sb[:])
                gp = stream.tile([HL, 512], F32, name="gp")
                nc.vector.tensor_scalar_add(gp[:], ge[:], 1.0)
                grec = stream.tile([HL, 512], F32, name="grec")
                nc.vector.reciprocal(grec[:], gp[:])
                nc.gpsimd.dma_start(
                    gates_sb[:, ci * 512:(ci + 1) * 512]
                    .rearrange("(a b) c -> a b c", b=32)[:, 0, :],
                    grec[:],
                )
                if dbg and ci == 0:
                    nc.gpsimd.dma_start(dbg["xnT"][:], xnT[:])

            # =========== Phase B: attention + out proj + RS ===========
            ydram = []
            rsout = []
            for ci in range(NQ):
                ydram.append(dramp.tile([QT, DIM], F32, name=f"ydram{ci}"))
                rsout.append(dramp.tile([P, DIM], F32, name=f"rsout{ci}"))

            if dbg:
                nc.gpsimd.dma_start(dbg["qkt"][:], QKT_sb[:])
                nc.gpsimd.dma_start(dbg["v"][:], v_sb[:])
                nc.gpsimd.dma_start(dbg["gates"][:], gates_sb[:, 0:512])

            nq_run = int(os.environ.get("KQUARTERS", NQ))
            for ci in range(nq_run):
                av01 = ps_qa1.tile([P, 512], F32, name="av01", tag="qa1")
                av23 = ps_qa2.tile([P, 512], F32, name="av23", tag="qa2")
                sums = ps_ygs.tile([97, 512], F32, name="sums", tag="ygs")
                islc = slice(ci * 512, (ci + 1) * 512)
                for jt in range(TT):
                    if (ci, jt) in pre_pts:
                        pts = pre_pts.pop((ci, jt))
                    else:
                        pts = emit_se(ci, jt)
                    emit_avs(jt, pts, av01, av23, sums)

                # prefetch the next quarter's first scores+exps so the ACT
                # queue has no gap across the quarter boundary
                if ci + 1 < nq_run:
                    for jt in range(4):
                        pre_pts[(ci + 1, jt)] = emit_se(ci + 1, jt)

                if dbg and ci == 0:
                    smd = tailp.tile([P, 512], F32, name="smd")
                    nc.vector.tensor_copy(smd[0:97, :], sums[0:97, :])
                    nc.gpsimd.dma_start(dbg["sums"][:], smd[:])
                    avd = tailp.tile([P, 512], F32, name="avd")
                    nc.vector.tensor_copy(avd[:], av01[:])
                    nc.gpsimd.dma_start(dbg["av"][:], avd[:])

                # normalize + gate -> outflatT. The per-query scale
                # c = gate/softmax_sum is broadcast across the 64 head dims
                # with a K=1 ones outer-product on the PE.
                oT = tailp.tile([P, 2, 512], BF16, name="oT")
                for hp in range(2):
                    c_ps = ps_s.tile([P, 512], F32, name="c_ps", tag="s")
                    for e in range(2):
                        h = 2 * hp + e
                        sr = tailp.tile([1, 512], F32, name="sr")
                        nc.vector.tensor_copy(sr[:], sums[h * 32:h * 32 + 1, :])
                        gr = tailp.tile([1, 512], F32, name="gr")
                        nc.vector.tensor_copy(gr[:], gates_sb[h * 32:h * 32 + 1, islc])
                        rc = tailp.tile([1, 512], F32, name="rc")
                        nc.vector.reciprocal(rc[:], sr[:])
                        cr = tailp.tile([1, 512], BF16, name="cr")
                        nc.vector.tensor_tensor(out=cr[:], in0=rc[:], in1=gr[:],
                                                op=ALU.mult)
                        nc.tensor.matmul(c_ps[e * 64:(e + 1) * 64, :],
                                         ones_row[:, :], cr[:],
                                         start=True, stop=True)
                    c_sb = tailp.tile([P, 512], F32, name="c_sb")
                    nc.vector.tensor_copy(c_sb[:], c_ps[:])
                    avt = av01 if hp == 0 else av23
                    nc.vector.tensor_tensor(out=oT[:, hp, :], in0=avt[:],
                                            in1=c_sb[:], op=ALU.mult)

                # output projection (partial over local heads)
                for tt in range(4):
                    for oc in range(2):
                        y_ps = ps_ygs.tile([P, 512], F32, name="y_ps", tag="ygs")
                        for kt in range(2):
                            nc.tensor.matmul(
                                y_ps[:],
                                oT[:, kt, tt * P:(tt + 1) * P],
                                wout_sb[:, kt, oc * 512:(oc + 1) * 512],
                                start=(kt == 0), stop=(kt == 1),
                            )
                        y_sb = tailp.tile([P, 512], F32, name="y_sb")
                        nc.vector.tensor_copy(y_sb[:], y_ps[:])
                        nc.sync.dma_start(
                            ydram[ci][tt * P:(tt + 1) * P,
                                      oc * 512:(oc + 1) * 512],
                            y_sb[:],
                        )

                if dbg and ci == 0:
                    nc.gpsimd.dma_start(dbg["oT"][:], oT[:])
                    nc.gpsimd.dma_start(dbg["y"][:], ydram[ci][:])

                if os.environ.get("KNOCOLL"):
                    nc.gpsimd.dma_start(out_ext[ci, :, :], ydram[ci][0:P, :])
                else:
                    nc.gpsimd.collective_compute(
                        "ReduceScatter", ALU.add,
                        replica_groups=REPLICA_GROUPS,
                        ins=[ydram[ci][:].opt()],
                        outs=[rsout[ci][:].opt()],
                    )
                    nc.sync.dma_start(out_ext[ci, :, :], rsout[ci][:])

    nc.compile()
    return nc


def _get_nc():
    global _nc_cache
    if _nc_cache is None:
        _nc_cache = _build()
    return _nc_cache


_PERM_EO = np.concatenate([np.arange(0, DH, 2), np.arange(1, DH, 2)])


def _shard(core, x, rotary_cos, rotary_sin, gamma, w_qkv, w_gates, b_gates, w_out):
    g, r = core // 4, core % 4
    heads = np.arange(4 * r, 4 * r + 4)
    wq = w_qkv[0 * DIM:1 * DIM] * gamma[None, :]
    wk = w_qkv[1 * DIM:2 * DIM] * gamma[None, :]
    wv = w_qkv[2 * DIM:3 * DIM]

    def qk_rows(w):
        # rows for local heads with even/odd permutation within each head
        idx = (heads[:, None] * DH + _PERM_EO[None, :]).reshape(-1)
        return w[idx]

    v_rows = wv[(heads[:, None] * DH + np.arange(DH)[None, :]).reshape(-1)]
    wqkv_t = np.concatenate([qk_rows(wq), qk_rows(wk), v_rows], axis=0).T
    wg_t = (w_gates[heads] * gamma[None, :]).T
    wout_t = w_out[:, heads[0] * DH:heads[0] * DH + HL * DH].T

    cos = rotary_cos[0, 0]  # (N, DH)
    sin = rotary_sin[0, 0]
    cosr = np.concatenate([np.tile(cos[:, 0::2], (1, 8)),
                           np.tile(cos[:, 1::2], (1, 8))], axis=1)
    sinr = np.concatenate([np.tile(sin[:, 0::2], (1, 8)),
                           np.tile(sin[:, 1::2], (1, 8))], axis=1)

    bf = ml_dtypes.bfloat16
    return {
        "x": np.ascontiguousarray(x[g], np.float32),
        "wqkv": np.ascontiguousarray(wqkv_t).astype(bf),
        "wg": np.ascontiguousarray(wg_t).astype(bf),
        "bgn": np.ascontiguousarray(-b_gates[heads].reshape(HL, 1), np.float32),
        "wout": np.ascontiguousarray(wout_t).astype(bf),
        "cosr": np.ascontiguousarray(cosr).astype(bf),
        "sinr": np.ascontiguousarray(sinr).astype(bf),
    }


def kernel(x, rotary_cos, rotary_sin, gamma, w_qkv, w_gates, b_gates, w_out):
    global _last_result
    args = [np.asarray(a, np.float32) for a in
            (x, rotary_cos, rotary_sin, gamma, w_qkv, w_gates, b_gates, w_out)]
    nc = _get_nc()
    in_maps = [_shard(c, *args) for c in range(CORES)]
    try:
        res = run_bass_kernel_spmd(
            nc, in_maps, core_ids=list(range(CORES)),
            trace=bool(os.environ.get("KTRACE")),
        )
    except ModuleNotFoundError:
        # profiler hook unavailable in this environment - run without trace
        res = run_bass_kernel_spmd(nc, in_maps, core_ids=list(range(CORES)))
    _last_result = res
    full = np.zeros((B, N, DIM), np.float32)
    for c in range(CORES):
        g, r = c // 4, c % 4
        o = np.asarray(res.results[c]["out"]).reshape(NQ, P, DIM)
        for q in range(NQ):
            full[g, q * 512 + r * P: q * 512 + (r + 1) * P, :] = o[q]
    return full



# revision 7
# speedup vs baseline: 1.3765x; 1.3765x over previous
"""Distributed Trainium2 kernel for nn_Attention_61332132987140.

Gated multi-head attention block: RMSNorm -> QKV proj -> RoPE -> softmax
attention -> sigmoid head gating -> output projection.

Sharding: 8 cores = 2 batch groups x 4-head groups (tensor parallel on
heads within a batch). Each core computes attention for its batch's full
sequence over its 4 heads, the partial output projection over its 256
rows of w_out, then a ReduceScatter over its 4-core batch group sums the
partials and writes each core's disjoint 128-token slice per 512-token
quarter directly into the output buffer. The host reassembles the full
(2, 2048, 1024) output.

Key cost-model-driven choices vs the v1 baseline:
- softmax denominators ride the AV matmuls as a 65th "ones" row of v
  (no separate PE sum-matmuls: -131k PE cycles)
- ReduceScatter writes straight into the external output (no bounce
  DRAM tile + DRAM->DRAM copy)
- out-proj PSUM is DMA'd to DRAM directly (no DVE copy)
- single ACT table set (Square/Ln/Exp/Copy) -> one table load
- RoPE via signed-sin + pair-swap strided views: 4 DVE ops/tile
- x and cos/sin are loaded in 4-tile batches to amortize DMA overhead
"""
import os
import sys

sys.path.insert(0, "/opt/trn_rl_repo")

import numpy as np
import ml_dtypes

import concourse.bass as bass
import concourse.mybir as mybir
import concourse.tile as tile
from concourse import bacc
from concourse.bass_utils import run_bass_kernel_spmd

F32 = mybir.dt.float32
BF16 = mybir.dt.bfloat16
AF = mybir.ActivationFunctionType
ALU = mybir.AluOpType

B, N, DIM = 2, 2048, 1024
HEADS, DH = 16, 64
HL = 4  # local heads per core
P = 128
TT = N // P  # 16 token tiles
KD = DIM // P  # 8 contraction tiles
NQ = 4  # quarters (512-token query chunks)
QT = N // NQ
CORES = 8
REPLICA_GROUPS = [[0, 1, 2, 3], [4, 5, 6, 7]]

_nc_cache = None
_last_result = None


def _build():
    nc = bacc.Bacc("TRN2", target_bir_lowering=False, debug=False, num_devices=CORES)

    x_ext = nc.declare_dram_parameter("x", [N, DIM], F32, isOutput=False)
    wqkv_ext = nc.declare_dram_parameter("wqkv", [DIM, 768], BF16, isOutput=False)
    wg_ext = nc.declare_dram_parameter("wg", [DIM, HL], BF16, isOutput=False)
    bgn_ext = nc.declare_dram_parameter("bgn", [HL, 1], F32, isOutput=False)
    wout_ext = nc.declare_dram_parameter("wout", [HL * DH, DIM], BF16, isOutput=False)
    cs_ext = nc.declare_dram_parameter("cs", [N, 1024], BF16, isOutput=False)
    out_ext = nc.declare_dram_parameter("out", [NQ, P, DIM], F32, isOutput=True)

    with tile.TileContext(nc) as tc:
        with (
            tc.tile_pool(name="wpool", bufs=1) as wpool,
            tc.tile_pool(name="persist", bufs=1) as persist,
            tc.tile_pool(name="xpool", bufs=2) as xpool,
            tc.tile_pool(name="cspool", bufs=2) as cspool,
            tc.tile_pool(name="stream", bufs=3) as stream,
            tc.tile_pool(name="scrp", bufs=2) as scrp,
            tc.tile_pool(name="xntp", bufs=3) as xntp,
            tc.tile_pool(name="rtmp", bufs=4) as rtmp,
            tc.tile_pool(name="ptp", bufs=16) as ptp,
            tc.tile_pool(name="tail", bufs=2) as tailp,
            tc.tile_pool(name="ps_s", bufs=2, space="PSUM") as ps_s,
            tc.tile_pool(name="ps_wk", bufs=4, space="PSUM") as ps_wk,
            tc.tile_pool(name="dram", bufs=1, space="DRAM") as dramp,
        ):
            # ---- constants / weights ----
            wqkv_sb = wpool.tile([P, KD, 768], BF16)
            nc.scalar.dma_start(
                wqkv_sb[:], wqkv_ext.rearrange("(k p) f -> p k f", p=P)
            )
            wg_sb = wpool.tile([P, KD, HL], BF16)
            nc.scalar.dma_start(wg_sb[:], wg_ext.rearrange("(k p) f -> p k f", p=P))
            wout_sb = wpool.tile([P, 2, DIM], BF16)
            nc.scalar.dma_start(
                wout_sb[:], wout_ext.rearrange("(k p) f -> p k f", p=P)
            )
            bgn_sb = wpool.tile([HL, 1], F32)
            nc.scalar.dma_start(bgn_sb[:], bgn_ext[:])
            zb = wpool.tile([P, 1], F32)
            nc.gpsimd.memset(zb[:], 0.0)
            ones_row = wpool.tile([1, 64], BF16)
            nc.gpsimd.memset(ones_row[:], 1.0)

            # ---- persistent activations ----
            # QKT_sb[p, tok_tile, blk, t]: blk 0/1 = q head-pairs; 2/3 = k.
            QKT_sb = persist.tile([P, TT, 4, P], BF16)
            # v1_sb[j_in_tile, jt, h*65 + d]; column h*65+64 is constant 1
            # so the AV matmul's 65th output row accumulates the softmax
            # denominator. memset everything to 1; v copies overwrite 0:64.
            v1_sb = persist.tile([P, TT, HL * 65], BF16)
            nc.gpsimd.memset(v1_sb[:], 1.0)
            # gates for head h live at partition 32*h; other rows garbage
            gates_sb = persist.tile([P, N], F32)

            pts = {}

            def emit_se(ci, jt):
                """scores + exp for one (quarter, j-tile)"""
                out = []
                for hp in range(2):
                    s_ps = ps_s.tile([P, 2, 512], F32, name="s_ps", tag="s")
                    for e in range(2):
                        nc.tensor.matmul(
                            s_ps[:, e, :],
                            QKT_sb[e * 64:(e + 1) * 64, jt, 2 + hp, :],
                            QKT_sb[e * 64:(e + 1) * 64, 4 * ci:4 * ci + 4,
                                   hp, :],
                            start=True, stop=True,
                        )
                    pt = ptp.tile([P, 2, 512], BF16, name="pt")
                    nc.scalar.activation(pt[:], s_ps[:], AF.Exp, scale=0.125,
                                         bias=zb[:])
                    out.append(pt)
                pts[(ci, jt)] = out

            def emit_avs(ci, jt, av):
                """AV (+denominator) accumulation for one j-tile"""
                p0, p1 = pts.pop((ci, jt))
                for h in range(HL):
                    hp, e = h // 2, h % 2
                    nc.tensor.matmul(
                        av[h][:],
                        v1_sb[:, jt, h * 65:(h + 1) * 65],
                        (p0 if hp == 0 else p1)[:, e, :],
                        start=(jt == 0), stop=(jt == TT - 1),
                        skip_group_check=True,
                    )

            # =========== Phase A: norm, QKV, RoPE, gates, transposes ========
            for ci in range(NQ):
                x4 = xpool.tile([P, 4, DIM], F32, name="x4")
                nc.gpsimd.dma_start(
                    x4[:],
                    x_ext[ci * 512:(ci + 1) * 512, :]
                    .rearrange("(t p) f -> p t f", p=P),
                )
                cs4 = cspool.tile([P, 4, 1024], BF16, name="cs4")
                nc.sync.dma_start(
                    cs4[:],
                    cs_ext[ci * 512:(ci + 1) * 512, :]
                    .rearrange("(t p) f -> p t f", p=P),
                )
                ss4 = stream.tile([P, 4], F32, name="ss4", tag="ss4")
                for tt in range(4):
                    scr = scrp.tile([P, DIM], BF16, name="scr", tag="scr")
                    nc.scalar.activation(scr[:], x4[:, tt, :], AF.Square,
                                         accum_out=ss4[:, tt:tt + 1])
                ln4 = stream.tile([P, 4], F32, name="ln4", tag="ln4")
                nc.scalar.activation(ln4[:], ss4[:], AF.Ln, bias=zb[:])
                # rs = 1/||x||ss; the *32*gamma factor is folded into weights
                rs4 = stream.tile([P, 4], F32, name="rs4", tag="rs4")
                nc.scalar.activation(rs4[:], ln4[:], AF.Exp, scale=-0.5,
                                     bias=zb[:])

                grq = stream.tile([HL, 512], F32, name="grq", tag="grq")
                for tt in range(4):
                    tok = ci * 4 + tt
                    xn_t = scrp.tile([P, DIM], BF16, name="xn_t", tag="xn_t")
                    nc.scalar.activation(xn_t[:], x4[:, tt, :], AF.Copy,
                                         scale=rs4[:, tt:tt + 1])
                    xnT = xntp.tile([P, KD, P], BF16, name="xnT")
                    nc.sync.dma_start_transpose(xnT[:], xn_t[:])

                    qk_ps = ps_wk.tile([P, 512], F32, name="qk_ps", tag="wk")
                    v_ps = ps_wk.tile([P, 256], F32, name="v_ps", tag="wk")
                    for kd in range(KD):
                        nc.tensor.matmul(qk_ps[:], xnT[:, kd, :],
                                         wqkv_sb[:, kd, 0:512],
                                         start=(kd == 0), stop=(kd == KD - 1))
                        nc.tensor.matmul(v_ps[:], xnT[:, kd, :],
                                         wqkv_sb[:, kd, 512:768],
                                         start=(kd == 0), stop=(kd == KD - 1))
                    g_ps = ps_wk.tile([HL, P], F32, name="g_ps", tag="wk")
                    for kd in range(KD):
                        nc.tensor.matmul(g_ps[:], wg_sb[:, kd, :],
                                         xnT[:, kd, :],
                                         start=(kd == 0), stop=(kd == KD - 1))

                    # RoPE: q' = q*cos + swap(q)*signed_sin, interleaved pairs
                    qv = qk_ps[:].rearrange("p (c two) -> p c two", two=2)
                    cos_f = cs4[:, tt, 0:512]
                    sv = cs4[:, tt, 512:1024].rearrange("p (c two) -> p c two",
                                                        two=2)
                    t_r = rtmp.tile([P, 512], F32, name="t_r", tag="t_r", bufs=2)
                    tv = t_r[:].rearrange("p (c two) -> p c two", two=2)
                    nc.vector.tensor_tensor(out=tv[:, :, 0], in0=qv[:, :, 1],
                                            in1=sv[:, :, 0], op=ALU.mult)
                    nc.vector.tensor_tensor(out=tv[:, :, 1], in0=qv[:, :, 0],
                                            in1=sv[:, :, 1], op=ALU.mult)
                    a_r = rtmp.tile([P, 512], F32, name="a_r", tag="a_r", bufs=2)
                    nc.vector.tensor_tensor(out=a_r[:], in0=qk_ps[:],
                                            in1=cos_f, op=ALU.mult)
                    qk_sb = rtmp.tile([P, 512], BF16, name="qk_sb", tag="qk_sb", bufs=2)
                    nc.vector.tensor_tensor(out=qk_sb[:], in0=a_r[:],
                                            in1=t_r[:], op=ALU.add)
                    nc.sync.dma_start_transpose(QKT_sb[:, tok, :, :], qk_sb[:])

                    # v (normalized scale rides the weights; rs applied via
                    # xn_t which fed the matmul)
                    nc.vector.tensor_copy(
                        v1_sb[:, tok, :]
                        .rearrange("p (h dv) -> p h dv", dv=65)[:, :, 0:64],
                        v_ps[:].rearrange("p (h dv) -> p h dv", dv=64),
                    )

                    # gates: sigmoid(z+b) = 1/(1+exp(-z-b))
                    ge = stream.tile([HL, P], F32, name="ge", tag="ge")
                    nc.scalar.activation(ge[:], g_ps[:], AF.Exp, scale=-1.0,
                                         bias=bgn_sb[:])
                    gp = stream.tile([HL, P], F32, name="gp", tag="gp")
                    nc.vector.tensor_scalar_add(gp[:], ge[:], 1.0)
                    nc.vector.reciprocal(grq[:, tt * P:(tt + 1) * P], gp[:])

                nc.gpsimd.dma_start(
                    gates_sb[:, ci * 512:(ci + 1) * 512]
                    .rearrange("(a b) c -> a b c", b=32)[:, 0, :],
                    grq[:],
                )
                # warm the score/exp pipeline during late phase A
                if ci == 1:
                    emit_se(0, 0)
                elif ci == 2:
                    emit_se(0, 1)
                    emit_se(0, 2)
                elif ci == 3:
                    emit_se(0, 3)

            # =========== Phase B: attention + out proj + RS ===========
            ydram = []
            rsout = []
            for ci in range(NQ):
                ydram.append(dramp.tile([QT, DIM], F32, name=f"ydram{ci}"))
                rsout.append(dramp.tile([P, DIM], F32, name=f"rsout{ci}"))

            for ci in range(NQ):
                av = []
                for h in range(HL):
                    av.append(ps_wk.tile([65, 512], F32, name=f"av{h}",
                                         tag="wk"))
                islc = slice(ci * 512, (ci + 1) * 512)
                for jt in range(TT):
                    if jt + 4 < TT:
                        emit_se(ci, jt + 4)
                    emit_avs(ci, jt, av)

                # normalize + gate: c = gate/denominator, broadcast across
                # the 64 head dims with a K=1 ones outer-product on the PE
                c2 = ps_s.tile([P, 2, 512], F32, name="c2", tag="s")
                for hp in range(2):
                    for e in range(2):
                        h = 2 * hp + e
                        rc = tailp.tile([1, 512], F32, name="rc", tag="rc")
                        nc.vector.reciprocal(rc[:], av[h][64:65, :])
                        gr = tailp.tile([1, 512], F32, name="gr", tag="gr")
                        nc.vector.tensor_copy(
                            gr[:], gates_sb[h * 32:h * 32 + 1, islc])
                        cr = tailp.tile([1, 512], BF16, name="cr", tag="cr")
                        nc.vector.tensor_tensor(
                            out=cr[:], in0=rc[:], in1=gr[:], op=ALU.mult)
                        nc.tensor.matmul(c2[e * 64:(e + 1) * 64, hp, :],
                                         ones_row[:, :], cr[:],
                                         start=True, stop=True)
                c_sb = tailp.tile([P, 2, 512], F32, name="c_sb", tag="c_sb")
                nc.vector.tensor_copy(c_sb[:], c2[:])
                oT = tailp.tile([P, 2, 512], BF16, name="oT", tag="oT")
                for hp in range(2):
                    for e in range(2):
                        h = 2 * hp + e
                        nc.vector.tensor_tensor(
                            out=oT[e * 64:(e + 1) * 64, hp, :],
                            in0=av[h][0:64, :],
                            in1=c_sb[e * 64:(e + 1) * 64, hp, :],
                            op=ALU.mult)

                # output projection (partial over local heads) + next
                # quarter's score prefetch interleaved
                for tt in range(4):
                    if ci + 1 < NQ:
                        emit_se(ci + 1, tt)
                    y2 = ps_s.tile([P, 2, 512], F32, name="y2", tag="s")
                    for oc in range(2):
                        for kt in range(2):
                            nc.tensor.matmul(
                                y2[:, oc, :],
                                oT[:, kt, tt * P:(tt + 1) * P],
                                wout_sb[:, kt, oc * 512:(oc + 1) * 512],
                                start=(kt == 0), stop=(kt == 1),
                            )
                    y_sb = tailp.tile([P, 2, 512], F32, name="y_sb", tag="y_sb", bufs=4)
                    nc.vector.tensor_copy(y_sb[:], y2[:])
                    nc.sync.dma_start(
                        ydram[ci][tt * P:(tt + 1) * P, :],
                        y_sb[:].rearrange("p a b -> p (a b)"),
                    )

                if os.environ.get("KNOCOLL"):
                    nc.gpsimd.dma_start(out_ext[ci, :, :], ydram[ci][0:P, :])
                else:
                    nc.gpsimd.collective_compute(
                        "ReduceScatter", ALU.add,
                        replica_groups=REPLICA_GROUPS,
                        ins=[ydram[ci][:].opt()],
                        outs=[rsout[ci][:].opt()],
                    )
                    # IO tensors can't be collective outputs; bounce through
                    # SBUF (two 1.6us DMAs) instead of a 12.6us DRAM->DRAM
                    ob = tailp.tile([P, DIM], F32, name="ob", tag="ob")
                    nc.gpsimd.dma_start(ob[:], rsout[ci][:])
                    nc.sync.dma_start(out_ext[ci, :, :], ob[:])

    nc.compile()
    return nc


def _get_nc():
    global _nc_cache
    if _nc_cache is None:
        _nc_cache = _build()
    return _nc_cache


def _shard(core, x, rotary_cos, rotary_sin, gamma, w_qkv, w_gates, b_gates, w_out):
    g, r = core // 4, core % 4
    heads = np.arange(4 * r, 4 * r + 4)
    # fold gamma and the sqrt(DIM)=32 norm factor into all input-side weights
    colscale = (gamma * 32.0)[None, :]
    wq = w_qkv[0 * DIM:1 * DIM] * colscale
    wk = w_qkv[1 * DIM:2 * DIM] * colscale
    wv = w_qkv[2 * DIM:3 * DIM] * colscale

    def rows(w):
        return w[(heads[:, None] * DH + np.arange(DH)[None, :]).reshape(-1)]

    wqkv_t = np.concatenate([rows(wq), rows(wk), rows(wv)], axis=0).T
    wg_t = (w_gates[heads] * colscale).T
    wout_t = w_out[:, heads[0] * DH:heads[0] * DH + HL * DH].T

    cos = rotary_cos[0, 0]  # (N, DH)
    sin = rotary_sin[0, 0]
    alt = np.where(np.arange(DH) % 2 == 0, -1.0, 1.0)[None, :]
    cs = np.concatenate([np.tile(cos, (1, 8)),
                         np.tile(sin * alt, (1, 8))], axis=1)

    bf = ml_dtypes.bfloat16
    return {
        "x": np.ascontiguousarray(x[g], np.float32),
        "wqkv": np.ascontiguousarray(wqkv_t).astype(bf),
        "wg": np.ascontiguousarray(wg_t).astype(bf),
        "bgn": np.ascontiguousarray(-b_gates[heads].reshape(HL, 1), np.float32),
        "wout": np.ascontiguousarray(wout_t).astype(bf),
        "cs": np.ascontiguousarray(cs).astype(bf),
    }


def kernel(x, rotary_cos, rotary_sin, gamma, w_qkv, w_gates, b_gates, w_out):
    global _last_result
    args = [np.asarray(a, np.float32) for a in
            (x, rotary_cos, rotary_sin, gamma, w_qkv, w_gates, b_gates, w_out)]
    nc = _get_nc()
    in_maps = [_shard(c, *args) for c in range(CORES)]
    try:
        res = run_bass_kernel_spmd(
            nc, in_maps, core_ids=list(range(CORES)),
            trace=bool(os.environ.get("KTRACE")),
        )
    except ModuleNotFoundError:
        # profiler hook unavailable in this environment - run without trace
        res = run_bass_kernel_spmd(nc, in_maps, core_ids=list(range(CORES)))
    _last_result = res
    full = np.zeros((B, N, DIM), np.float32)
    for c in range(CORES):
        g, r = c // 4, c % 4
        o = np.asarray(res.results[c]["out"]).reshape(NQ, P, DIM)
        for q in range(NQ):
            full[g, q * 512 + r * P: q * 512 + (r + 1) * P, :] = o[q]
    return full


# revision 10
# speedup vs baseline: 1.4089x; 1.0236x over previous
"""Distributed Trainium2 kernel for nn_Attention_61332132987140.

Gated multi-head attention block: RMSNorm -> QKV proj -> RoPE -> softmax
attention -> sigmoid head gating -> output projection.

Sharding: 8 cores = 2 batch groups x 4-head groups (tensor parallel on
heads within a batch). Each core computes attention for its batch's full
sequence over its 4 heads, the partial output projection over its 256
rows of w_out, then a ReduceScatter over its 4-core batch group sums the
partials; each core's 128-token shard per 512-token quarter is bounced
through SBUF into the output buffer. Host reassembles (2, 2048, 1024).

Cost-model-driven structure (CoreSim timing is the grading metric):
- softmax denominators ride the AV matmuls as a 65th "ones" row of v
- gate logits ride the QKV projection as 4 extra weight columns; the
  [128,4] -> [4,128] flip is a PE-transpose against an identity
- 1/sqrt(ss) via bit-trick + Newton on DVE: the ACT queue then only
  ever uses the exp-table function set (Square/Exp/Copy) -> one
  LoadActFuncSet total instead of 9
- quarter boundaries: next quarter's score/exp prefetch is emitted
  before the gate/normalize chain so ACT never drains; gate rows are
  pre-copied at quarter start
- x and cos/sin are loaded in 4-tile batches to amortize DMA overhead
"""
import os
import sys

sys.path.insert(0, "/opt/trn_rl_repo")

import numpy as np
import ml_dtypes

import concourse.bass as bass
import concourse.mybir as mybir
import concourse.tile as tile
from concourse import bacc
from concourse.bass_utils import run_bass_kernel_spmd

F32 = mybir.dt.float32
BF16 = mybir.dt.bfloat16
I32 = mybir.dt.int32
AF = mybir.ActivationFunctionType
ALU = mybir.AluOpType

B, N, DIM = 2, 2048, 1024
HEADS, DH = 16, 64
HL = 4  # local heads per core
P = 128
TT = N // P  # 16 token tiles
KD = DIM // P  # 8 contraction tiles
NQ = 4  # quarters (512-token query chunks)
QT = N // NQ
CORES = 8
REPLICA_GROUPS = [[0, 1, 2, 3], [4, 5, 6, 7]]
RSQRT_MAGIC = 0x5F3759DF

_nc_cache = None
_last_result = None


def _build():
    nc = bacc.Bacc("TRN2", target_bir_lowering=False, debug=False, num_devices=CORES)

    x_ext = nc.declare_dram_parameter("x", [N, DIM], BF16, isOutput=False)
    wqkv_ext = nc.declare_dram_parameter("wqkv", [DIM, 772], BF16, isOutput=False)
    bgn_ext = nc.declare_dram_parameter("bgn", [HL, 1], F32, isOutput=False)
    wout_ext = nc.declare_dram_parameter("wout", [HL * DH, DIM], BF16, isOutput=False)
    cs_ext = nc.declare_dram_parameter("cs", [N, 1024], BF16, isOutput=False)
    ident_ext = nc.declare_dram_parameter("ident", [P, P], BF16, isOutput=False)
    out_ext = nc.declare_dram_parameter("out", [NQ, P, DIM], F32, isOutput=True)

    with tile.TileContext(nc) as tc:
        with (
            tc.tile_pool(name="wpool", bufs=1) as wpool,
            tc.tile_pool(name="persist", bufs=1) as persist,
            tc.tile_pool(name="xpool", bufs=2) as xpool,
            tc.tile_pool(name="cspool", bufs=2) as cspool,
            tc.tile_pool(name="stream", bufs=3) as stream,
            tc.tile_pool(name="scrp", bufs=2) as scrp,
            tc.tile_pool(name="xntp", bufs=3) as xntp,
            tc.tile_pool(name="rtmp", bufs=2) as rtmp,
            tc.tile_pool(name="ptp", bufs=16) as ptp,
            tc.tile_pool(name="tail", bufs=2) as tailp,
            tc.tile_pool(name="ps_s", bufs=2, space="PSUM") as ps_s,
            tc.tile_pool(name="ps_wk", bufs=4, space="PSUM") as ps_wk,
            tc.tile_pool(name="dram", bufs=1, space="DRAM") as dramp,
        ):
            # ---- constants / weights ----
            wqkv_sb = wpool.tile([P, KD, 772], BF16)
            nc.scalar.dma_start(
                wqkv_sb[:], wqkv_ext.rearrange("(k p) f -> p k f", p=P)
            )
            wout_sb = wpool.tile([P, 2, DIM], BF16)
            nc.scalar.dma_start(
                wout_sb[:], wout_ext.rearrange("(k p) f -> p k f", p=P)
            )
            bgn_sb = wpool.tile([HL, 1], F32)
            nc.scalar.dma_start(bgn_sb[:], bgn_ext[:])
            ident_sb = wpool.tile([P, P], BF16)
            nc.scalar.dma_start(ident_sb[:], ident_ext[:])
            zb = wpool.tile([P, 1], F32)
            nc.gpsimd.memset(zb[:], 0.0)
            ones_row = wpool.tile([1, 64], BF16)
            nc.gpsimd.memset(ones_row[:], 1.0)
            magic_sb = wpool.tile([P, 4], I32)
            nc.gpsimd.memset(magic_sb[:], RSQRT_MAGIC)
            c15_sb = wpool.tile([P, 4], F32)
            nc.gpsimd.memset(c15_sb[:], 1.5)

            # ---- persistent activations ----
            # QKT_sb[p, tok_tile, blk, t]: blk 0/1 = q head-pairs; 2/3 = k.
            QKT_sb = persist.tile([P, TT, 4, P], BF16)
            # v1_sb[j_in_tile, jt, h*65 + d]; column h*65+64 stays 1 from the
            # memset so the AV matmul's 65th row accumulates the softmax
            # denominator.
            v1_sb = persist.tile([P, TT, HL * 65], BF16)
            nc.gpsimd.memset(v1_sb[:], 1.0)
            # gates for head h live at partition 32*h; other rows garbage
            gates_sb = persist.tile([P, N], F32)

            pts = {}
            se_next = [0] * NQ

            def emit_se(ci, jt):
                """scores + exp for one (quarter, j-tile)"""
                out = []
                for hp in range(2):
                    s_ps = ps_s.tile([P, 2, 512], F32, name="s_ps", tag="s")
                    for e in range(2):
                        nc.tensor.matmul(
                            s_ps[:, e, :],
                            QKT_sb[e * 64:(e + 1) * 64, jt, 2 + hp, :],
                            QKT_sb[e * 64:(e + 1) * 64, 4 * ci:4 * ci + 4,
                                   hp, :],
                            start=True, stop=True,
                        )
                    pt = ptp.tile([P, 2, 512], BF16, name="pt")
                    nc.scalar.activation(pt[:], s_ps[:], AF.Exp, scale=0.125,
                                         bias=zb[:])
                    out.append(pt)
                pts[(ci, jt)] = out

            def pump(ci, upto):
                while se_next[ci] <= min(upto, TT - 1):
                    emit_se(ci, se_next[ci])
                    se_next[ci] += 1

            def emit_avs(ci, jt, av):
                """AV (+denominator) accumulation for one j-tile"""
                p0, p1 = pts.pop((ci, jt))
                for h in range(HL):
                    hp, e = h // 2, h % 2
                    nc.tensor.matmul(
                        av[h][:],
                        v1_sb[:, jt, h * 65:(h + 1) * 65],
                        (p0 if hp == 0 else p1)[:, e, :],
                        start=(jt == 0), stop=(jt == TT - 1),
                        skip_group_check=True,
                    )

            # =========== Phase A: norm, QKV, RoPE, gates, transposes ========
            for ci in range(NQ):
                x4 = xpool.tile([P, 4, DIM], BF16, name="x4")
                nc.gpsimd.dma_start(
                    x4[:],
                    x_ext[ci * 512:(ci + 1) * 512, :]
                    .rearrange("(t p) f -> p t f", p=P),
                )
                cs4 = cspool.tile([P, 4, 1024], BF16, name="cs4")
                nc.sync.dma_start(
                    cs4[:],
                    cs_ext[ci * 512:(ci + 1) * 512, :]
                    .rearrange("(t p) f -> p t f", p=P),
                )
                ss4 = stream.tile([P, 4], F32, name="ss4", tag="ss4")
                for tt in range(4):
                    scr = scrp.tile([P, DIM], BF16, name="scr", tag="scr")
                    nc.scalar.activation(scr[:], x4[:, tt, :], AF.Square,
                                         accum_out=ss4[:, tt:tt + 1])
                # rs4 = 1/sqrt(ss4): bit-trick + 2 Newton steps on DVE. This
                # keeps ACT on the exp-table set (no LoadActFuncSet churn).
                # *32*gamma is folded into the weights host-side.
                t1 = stream.tile([P, 4], I32, name="t1", tag="t1")
                nc.vector.tensor_scalar(out=t1[:], in0=ss4[:].bitcast(I32),
                                        scalar1=1, scalar2=None,
                                        op0=ALU.arith_shift_right)
                y0 = stream.tile([P, 4], F32, name="y0", tag="y0")
                nc.vector.tensor_tensor(out=y0[:].bitcast(I32),
                                        in0=magic_sb[:], in1=t1[:],
                                        op=ALU.subtract)
                h4 = stream.tile([P, 4], F32, name="h4", tag="h4")
                nc.vector.tensor_scalar_mul(h4[:], ss4[:], 0.5)
                yc = y0
                for it in range(2):
                    sq = stream.tile([P, 4], F32, name="sq", tag=f"sq{it}")
                    nc.vector.tensor_tensor(out=sq[:], in0=yc[:], in1=yc[:],
                                            op=ALU.mult)
                    hq = stream.tile([P, 4], F32, name="hq", tag=f"hq{it}")
                    nc.vector.tensor_tensor(out=hq[:], in0=h4[:], in1=sq[:],
                                            op=ALU.mult)
                    u = stream.tile([P, 4], F32, name="u", tag=f"u{it}")
                    nc.vector.tensor_tensor(out=u[:], in0=c15_sb[:],
                                            in1=hq[:], op=ALU.subtract)
                    yn = stream.tile([P, 4], F32, name="yn", tag=f"yn{it}")
                    nc.vector.tensor_tensor(out=yn[:], in0=yc[:], in1=u[:],
                                            op=ALU.mult)
                    yc = yn
                rs4 = yc

                grq = stream.tile([HL, 512], F32, name="grq", tag="grq")
                for tt in range(4):
                    tok = ci * 4 + tt
                    xn_t = scrp.tile([P, DIM], BF16, name="xn_t", tag="xn_t")
                    nc.scalar.activation(xn_t[:], x4[:, tt, :], AF.Copy,
                                         scale=rs4[:, tt:tt + 1])
                    xnT = xntp.tile([P, KD, P], BF16, name="xnT")
                    nc.sync.dma_start_transpose(xnT[:], xn_t[:])

                    qk_ps = ps_wk.tile([P, 512], F32, name="qk_ps", tag="wk")
                    vg_ps = ps_wk.tile([P, 260], F32, name="vg_ps", tag="wk")
                    for kd in range(KD):
                        nc.tensor.matmul(qk_ps[:], xnT[:, kd, :],
                                         wqkv_sb[:, kd, 0:512],
                                         start=(kd == 0), stop=(kd == KD - 1))
                    for kd in range(KD):
                        nc.tensor.matmul(vg_ps[:], xnT[:, kd, :],
                                         wqkv_sb[:, kd, 512:772],
                                         start=(kd == 0), stop=(kd == KD - 1))

                    # RoPE: q' = q*cos + swap(q)*signed_sin, interleaved pairs
                    qv = qk_ps[:].rearrange("p (c two) -> p c two", two=2)
                    cos_f = cs4[:, tt, 0:512]
                    sv = cs4[:, tt, 512:1024].rearrange("p (c two) -> p c two",
                                                        two=2)
                    t_r = rtmp.tile([P, 512], F32, name="t_r", tag="t_r")
                    tv = t_r[:].rearrange("p (c two) -> p c two", two=2)
                    nc.vector.tensor_tensor(out=tv[:, :, 0], in0=qv[:, :, 1],
                                            in1=sv[:, :, 0], op=ALU.mult)
                    nc.vector.tensor_tensor(out=tv[:, :, 1], in0=qv[:, :, 0],
                                            in1=sv[:, :, 1], op=ALU.mult)
                    a_r = rtmp.tile([P, 512], F32, name="a_r", tag="a_r")
                    nc.vector.tensor_tensor(out=a_r[:], in0=qk_ps[:],
                                            in1=cos_f, op=ALU.mult)
                    qk_sb = rtmp.tile([P, 512], BF16, name="qk_sb",
                                      tag="qk_sb")
                    nc.vector.tensor_tensor(out=qk_sb[:], in0=a_r[:],
                                            in1=t_r[:], op=ALU.add)
                    nc.sync.dma_start_transpose(QKT_sb[:, tok, :, :], qk_sb[:])

                    # v into the 65-strided AV layout
                    nc.vector.tensor_copy(
                        v1_sb[:, tok, :]
                        .rearrange("p (h dv) -> p h dv", dv=65)[:, :, 0:64],
                        vg_ps[:, 0:256].rearrange("p (h dv) -> p h dv", dv=64),
                    )

                    # gates: flip [128,4] -> [4,128] on the PE, then
                    # sigmoid(z+b) = 1/(1+exp(-z-b))
                    gz = stream.tile([P, 4], BF16, name="gz", tag="gz")
                    nc.vector.tensor_copy(gz[:], vg_ps[:, 256:260])
                    gzT = ps_wk.tile([HL, P], BF16, name="gzT", tag="wk")
                    nc.tensor.transpose(gzT[:], gz[:], ident_sb[:])
                    ge = stream.tile([HL, P], F32, name="ge", tag="ge")
                    nc.scalar.activation(ge[:], gzT[:], AF.Exp, scale=-1.0,
                                         bias=bgn_sb[:])
                    gp = stream.tile([HL, P], F32, name="gp", tag="gp")
                    nc.vector.tensor_scalar_add(gp[:], ge[:], 1.0)
                    nc.vector.reciprocal(grq[:, tt * P:(tt + 1) * P], gp[:])

                nc.gpsimd.dma_start(
                    gates_sb[:, ci * 512:(ci + 1) * 512]
                    .rearrange("(a b) c -> a b c", b=32)[:, 0, :],
                    grq[:],
                )
                # warm the score/exp pipeline during late phase A
                if ci == 1:
                    pump(0, 0)
                elif ci == 2:
                    pump(0, 2)
                elif ci == 3:
                    pump(0, 5)

            # =========== Phase B: attention + out proj + RS ===========
            ydram = []
            rsout = []
            for ci in range(NQ):
                ydram.append(dramp.tile([QT, DIM], F32, name=f"ydram{ci}"))
                rsout.append(dramp.tile([P, DIM], F32, name=f"rsout{ci}"))

            for ci in range(NQ):
                av = []
                for h in range(HL):
                    av.append(ps_wk.tile([65, 512], F32, name=f"av{h}",
                                         tag="wk"))
                islc = slice(ci * 512, (ci + 1) * 512)
                # pre-copy gate rows while DVE is idle (they only depend on
                # phase A)
                grs = []
                for h in range(HL):
                    gr = tailp.tile([1, 512], BF16, name="gr", tag=f"gr{h}")
                    nc.vector.tensor_copy(
                        gr[:], gates_sb[h * 32:h * 32 + 1, islc])
                    grs.append(gr)
                for jt in range(TT):
                    pump(ci, jt + 4)
                    emit_avs(ci, jt, av)

                # keep ACT fed across the boundary before the gate chain
                if ci + 1 < NQ:
                    pump(ci + 1, 1)

                # c = gate/denominator per (head, query), broadcast across
                # the 64 head dims with a K=1 ones outer-product on the PE
                c2 = ps_s.tile([P, 2, 512], F32, name="c2", tag="s")
                for hp in range(2):
                    for e in range(2):
                        h = 2 * hp + e
                        rc = tailp.tile([1, 512], F32, name="rc", tag="rc")
                        nc.vector.reciprocal(rc[:], av[h][64:65, :])
                        cr = tailp.tile([1, 512], BF16, name="cr", tag="cr")
                        nc.vector.tensor_tensor(out=cr[:], in0=rc[:],
                                                in1=grs[h][:], op=ALU.mult)
                        nc.tensor.matmul(c2[e * 64:(e + 1) * 64, hp, :],
                                         ones_row[:, :], cr[:],
                                         start=True, stop=True)
                c_sb = tailp.tile([P, 2, 512], F32, name="c_sb", tag="c_sb")
                nc.vector.tensor_copy(c_sb[:], c2[:])
                oT = tailp.tile([P, 2, 512], BF16, name="oT", tag="oT")
                for hp in range(2):
                    for e in range(2):
                        h = 2 * hp + e
                        nc.vector.tensor_tensor(
                            out=oT[e * 64:(e + 1) * 64, hp, :],
                            in0=av[h][0:64, :],
                            in1=c_sb[e * 64:(e + 1) * 64, hp, :],
                            op=ALU.mult)

                # output projection (partial over local heads), next
                # quarter's score prefetch interleaved
                for tt in range(4):
                    y2 = ps_s.tile([P, 2, 512], F32, name="y2", tag="s")
                    for oc in range(2):
                        for kt in range(2):
                            nc.tensor.matmul(
                                y2[:, oc, :],
                                oT[:, kt, tt * P:(tt + 1) * P],
                                wout_sb[:, kt, oc * 512:(oc + 1) * 512],
                                start=(kt == 0), stop=(kt == 1),
                            )
                    y_sb = tailp.tile([P, 2, 512], F32, name="y_sb",
                                      tag="y_sb", bufs=2)
                    nc.vector.tensor_copy(y_sb[:], y2[:])
                    nc.sync.dma_start(
                        ydram[ci][tt * P:(tt + 1) * P, :],
                        y_sb[:].rearrange("p a b -> p (a b)"),
                    )
                    if ci + 1 < NQ:
                        pump(ci + 1, tt)

                if os.environ.get("KNOCOLL"):
                    nc.gpsimd.dma_start(out_ext[ci, :, :], ydram[ci][0:P, :])
                else:
                    nc.gpsimd.collective_compute(
                        "ReduceScatter", ALU.add,
                        replica_groups=REPLICA_GROUPS,
                        ins=[ydram[ci][:].opt()],
                        outs=[rsout[ci][:].opt()],
                    )
                    # IO tensors can't be collective outputs; bounce through
                    # SBUF (two fast DMAs) instead of a DRAM->DRAM copy
                    ob = tailp.tile([P, DIM], F32, name="ob", tag="ob", bufs=1)
                    nc.gpsimd.dma_start(ob[:], rsout[ci][:])
                    nc.sync.dma_start(out_ext[ci, :, :], ob[:])

    nc.compile()
    return nc


def _get_nc():
    global _nc_cache
    if _nc_cache is None:
        _nc_cache = _build()
    return _nc_cache


def _shard(core, x, rotary_cos, rotary_sin, gamma, w_qkv, w_gates, b_gates, w_out):
    g, r = core // 4, core % 4
    heads = np.arange(4 * r, 4 * r + 4)
    # fold gamma and the sqrt(DIM)=32 norm factor into all input-side weights
    colscale = (gamma * 32.0)[None, :]
    wq = w_qkv[0 * DIM:1 * DIM] * colscale
    wk = w_qkv[1 * DIM:2 * DIM] * colscale
    wv = w_qkv[2 * DIM:3 * DIM] * colscale

    def rows(w):
        return w[(heads[:, None] * DH + np.arange(DH)[None, :]).reshape(-1)]

    wg_rows = w_gates[heads] * colscale  # (4, DIM)
    wqkv_t = np.concatenate([rows(wq), rows(wk), rows(wv), wg_rows],
                            axis=0).T  # (DIM, 772)
    wout_t = w_out[:, heads[0] * DH:heads[0] * DH + HL * DH].T

    cos = rotary_cos[0, 0]  # (N, DH)
    sin = rotary_sin[0, 0]
    alt = np.where(np.arange(DH) % 2 == 0, -1.0, 1.0)[None, :]
    cs = np.concatenate([np.tile(cos, (1, 8)),
                         np.tile(sin * alt, (1, 8))], axis=1)

    bf = ml_dtypes.bfloat16
    return {
        "x": np.ascontiguousarray(x[g]).astype(bf),
        "wqkv": np.ascontiguousarray(wqkv_t).astype(bf),
        "bgn": np.ascontiguousarray(-b_gates[heads].reshape(HL, 1), np.float32),
        "wout": np.ascontiguousarray(wout_t).astype(bf),
        "cs": np.ascontiguousarray(cs).astype(bf),
        "ident": np.eye(P, dtype=np.float32).astype(bf),
    }


def kernel(x, rotary_cos, rotary_sin, gamma, w_qkv, w_gates, b_gates, w_out):
    global _last_result
    args = [np.asarray(a, np.float32) for a in
            (x, rotary_cos, rotary_sin, gamma, w_qkv, w_gates, b_gates, w_out)]
    nc = _get_nc()
    in_maps = [_shard(c, *args) for c in range(CORES)]
    try:
        res = run_bass_kernel_spmd(
            nc, in_maps, core_ids=list(range(CORES)),
            trace=bool(os.environ.get("KTRACE")),
        )
    except ModuleNotFoundError:
        # profiler hook unavailable in this environment - run without trace
        res = run_bass_kernel_spmd(nc, in_maps, core_ids=list(range(CORES)))
    _last_result = res
    full = np.zeros((B, N, DIM), np.float32)
    for c in range(CORES):
        g, r = c // 4, c % 4
        o = np.asarray(res.results[c]["out"]).reshape(NQ, P, DIM)
        for q in range(NQ):
            full[g, q * 512 + r * P: q * 512 + (r + 1) * P, :] = o[q]
    return full
